# revision 11
# baseline (speedup 1.0000x reference)
"""Multi-head attention (B=2, S=2048, D=1024, H=16) on 8 Trainium2 NeuronCores.

Sharding: core c handles (batch b=c//4, chunk j=c%4 of 512 rows).
 - ALL projections (Q, K, V) are computed only for the core's own 512 rows
   (no redundant full-batch K/V compute).
 - K^T / packed-V shards are exchanged within each batch group of 4 cores
   via ONE HBM AllGather (replica groups [[0,1,2,3],[4,5,6,7]]) that runs on
   the TOPSP/SDMA silicon, overlapped with the Q projection.
 - Scores are row-tiled: head-even uses PE array rows 0-63, head-odd rows
   64-127 concurrently (K=64 contraction per head), into separate PSUM
   tiles psA/psB of [128, 1024] (2 key-chunks each) -> one [128,1024] Exp
   per half on ACT (the ACT exp stream is the kernel's critical path).
 - PV uses the 65-wide [V|1] stationary trick for the softmax denominator:
   even heads pack [V(64) | 1], odd heads pack [1 | V(64)] so the odd
   head's attnT lands on PSUM partitions 64..127 (partition-aligned with
   its attn_sb slot) with Z on row 63.
 - Output projection interleaved into the attention stream; partials
   accumulate into SBUF f32 via DVE; out bias pre-broadcast by GPSIMD.
"""

import numpy as np
import ml_dtypes

import concourse.bass as bass
import concourse.mybir as mybir
import concourse.tile as tile
from concourse import bacc
from concourse.bass_utils import run_bass_kernel_spmd

BF16 = mybir.dt.bfloat16
F32 = mybir.dt.float32
AF = mybir.ActivationFunctionType

B, S, D = 2, 2048, 1024
H, HD = 16, 64
N_CORES = 8
R = 4            # cores per batch (gather group)
SL = S // R      # rows per core (queries AND keys/values projected here)
P = 128
DCH = D // P     # 8 d-chunks (contraction tiles)
NKK = S // P     # 16 key chunks of 128
ET = D // P      # 8 feature tiles per projection
FREE = 512
PAIRS = H // 2   # 8 head pairs
# V pack: per key-chunk, 16 heads x (64 V-features + ones col) at stride 65
# plus 64 zero tail cols so stationary slice [65h : 65h+128] stays in bounds
VW = H * (HD + 1) + HD        # 1104
KSEC = ET * FREE              # 4096 cols of K^T in the gather blob
BLOBW = KSEC + 4 * VW         # 8512
GROUPS = [[0, 1, 2, 3], [4, 5, 6, 7]]


def build_program():
    nc = bacc.Bacc("TRN2", target_bir_lowering=False, debug=False,
                   num_devices=N_CORES)

    xqT = nc.dram_tensor("xqT", [D, SL], BF16, kind="ExternalInput")
    wqkvT = nc.dram_tensor("wqkvT", [D, 3 * D], BF16, kind="ExternalInput")
    bqk = nc.dram_tensor("bqk", [P, 16], BF16, kind="ExternalInput")
    woutT = nc.dram_tensor("woutT", [D, D], BF16, kind="ExternalInput")
    bout = nc.dram_tensor("bout", [1, D], F32, kind="ExternalInput")
    out = nc.dram_tensor("out", [SL, D], F32, kind="ExternalOutput")

    kv_in = nc.dram_tensor("kv_in", [P, BLOBW], BF16)
    kv_out = nc.dram_tensor("kv_out", [R * P, BLOBW], BF16)

    with tile.TileContext(nc) as tc:
        _build(nc, tc, xqT, wqkvT, bqk, woutT, bout, out, kv_in, kv_out)
    nc.compile()
    return nc


def _build(nc, tc, xqT, wqkvT, bqk, woutT, bout, out, kv_in, kv_out):
    from contextlib import ExitStack

    ctx = ExitStack()
    consts = ctx.enter_context(tc.tile_pool(name="consts", bufs=1))

    bqk_sb = consts.tile([P, 16], BF16, name="bqk_sb")
    nc.sync.dma_start(bqk_sb[:], bqk.ap())
    bout_sb = consts.tile([1, D], F32, name="bout_sb")
    nc.sync.dma_start(bout_sb[:], bout.ap())

    # ---- own x chunk, transposed: [d, s_own] ----
    xq_pool = ctx.enter_context(tc.tile_pool(name="xq", bufs=1))
    xq = []
    for i in range(DCH):
        t = xq_pool.tile([P, SL], BF16, name=f"xq{i}")
        nc.sync.dma_start(t[:], xqT.ap()[P * i:P * (i + 1), :])
        xq.append(t)

    # ---- all qkv weights resident: wblk[(eb, d)] = [128 d, 512 e] ----
    # load order: K blocks (eb 2,3), V (4,5), Q (0,1) to match compute order
    w_pool = ctx.enter_context(tc.tile_pool(name="wq", bufs=1))
    wblk = {}
    for eb in (2, 3, 4, 5, 0, 1):
        for d in range(DCH):
            t = w_pool.tile([P, FREE], BF16, name=f"w{eb}_{d}")
            nc.gpsimd.dma_start(t[:], wqkvT.ap()[P * d:P * (d + 1),
                                                 FREE * eb:FREE * (eb + 1)])
            wblk[(eb, d)] = t

    # ---- K^T / V projections for the own 512 rows, staged to the gather
    # blob via transient SBUF tiles (pools close before attention pools
    # open so their SBUF is reused) ----
    with tc.tile_pool(name="kto", bufs=1) as kto_pool, \
         tc.tile_pool(name="vto", bufs=1) as vto_pool:
        kt_own = [kto_pool.tile([P, FREE], BF16, name=f"kto{t}")
                  for t in range(ET)]
        vt_own = [vto_pool.tile([P, VW], BF16, name=f"vto{t}")
                  for t in range(4)]
        with tc.tile_pool(name="projk_ps", bufs=4, space="PSUM") as ps_pool:
            for t in range(ET):
                eb = 2 + t // 4
                co = P * (t % 4)
                ps = ps_pool.tile([P, FREE], F32, name=f"psk{t}", tag="proj")
                for d in range(DCH):
                    nc.tensor.matmul(ps[:], wblk[(eb, d)][:, co:co + P],
                                     xq[d][:],
                                     start=(d == 0), stop=(d == DCH - 1))
                nc.scalar.activation(kt_own[t][:], ps[:], AF.Identity,
                                     bias=bqk_sb[:, 8 + t:9 + t])
                nc.sync.dma_start(kv_in.ap()[:, FREE * t:FREE * (t + 1)],
                                  kt_own[t][:])

        for t in range(4):
            v3 = vt_own[t][:, 0:H * (HD + 1)].rearrange("p (h c) -> p h c",
                                                        c=HD + 1)
            nc.vector.memset(v3[:, :, HD:HD + 1], 1.0)
            nc.vector.memset(vt_own[t][:, H * (HD + 1):VW], 0.0)
        with tc.tile_pool(name="projv_ps", bufs=4, space="PSUM") as ps_pool:
            for st in range(4):
                v3 = vt_own[st][:, 0:H * (HD + 1)].rearrange(
                    "p (h c) -> p h c", c=HD + 1)
                for eb in range(2):
                    ps = ps_pool.tile([P, FREE], F32, name=f"psv{st}_{eb}",
                                      tag="proj")
                    for d in range(DCH):
                        nc.tensor.matmul(ps[:], xq[d][:, P * st:P * (st + 1)],
                                         wblk[(4 + eb, d)][:],
                                         start=(d == 0), stop=(d == DCH - 1))
                    nc.vector.tensor_copy(
                        v3[:, 8 * eb:8 * (eb + 1), 0:HD],
                        ps.rearrange("p (h v) -> p h v", v=HD))
                nc.sync.dma_start(
                    kv_in.ap()[:, KSEC + VW * st:KSEC + VW * (st + 1)],
                    vt_own[st][:])

        # ---- gather K/V shards within the 4-core batch group ----
        nc.gpsimd.collective_compute(
            "AllGather", mybir.AluOpType.bypass, replica_groups=GROUPS,
            ins=[kv_in.ap().opt()], outs=[kv_out.ap().opt()])

    # ---- Q projection (runs on PE while the collective is in flight) ----
    qp_pool = ctx.enter_context(tc.tile_pool(name="qp", bufs=1))
    qp = [qp_pool.tile([P, FREE], BF16, name=f"qp{t}") for t in range(ET)]
    with tc.tile_pool(name="projq_ps", bufs=4, space="PSUM") as ps_pool:
        for t in range(ET):
            eb = t // 4
            co = P * (t % 4)
            ps = ps_pool.tile([P, FREE], F32, name=f"psq{t}", tag="proj")
            for d in range(DCH):
                nc.tensor.matmul(ps[:], wblk[(eb, d)][:, co:co + P], xq[d][:],
                                 start=(d == 0), stop=(d == DCH - 1))
            nc.scalar.activation(qp[t][:], ps[:], AF.Identity,
                                 bias=bqk_sb[:, t:t + 1])

    # ---- output-projection weights + osb init (bias broadcast) ----
    wo_pool = ctx.enter_context(tc.tile_pool(name="wo", bufs=1))
    wo = []
    for p_ in range(DCH):
        t = wo_pool.tile([P, D], BF16, name=f"wo{p_}")
        nc.sync.dma_start(t[:], woutT.ap()[P * p_:P * (p_ + 1), :])
        wo.append(t)
    osb_pool = ctx.enter_context(tc.tile_pool(name="osb", bufs=1))
    osb = [osb_pool.tile([P, D], F32, name=f"osb{st}")
           for st in range(SL // P)]
    for st in range(SL // P):
        nc.gpsimd.partition_broadcast(osb[st][:], bout_sb[:])

    # ---- gathered K^T / V tiles ----
    kt_pool = ctx.enter_context(tc.tile_pool(name="ktp", bufs=1))
    kt_pair = [kt_pool.tile([P, S], BF16, name=f"ktp{p}")
               for p in range(PAIRS)]
    vt_pool = ctx.enter_context(tc.tile_pool(name="vt", bufs=1))
    vt = [vt_pool.tile([P, VW], BF16, name=f"vt{c}") for c in range(NKK)]
    # interleave per-pair K cols and V tiles so early attention pairs get
    # their data first
    for p in range(PAIRS):
        for r in range(R):
            nc.sync.dma_start(
                kt_pair[p][:, FREE * r:FREE * (r + 1)],
                kv_out.ap()[P * r:P * (r + 1), FREE * p:FREE * (p + 1)])
        if p < 4:
            for r in range(R):
                c = 4 * r + p
                nc.sync.dma_start(
                    vt[c][:],
                    kv_out.ap()[P * r:P * (r + 1),
                                KSEC + VW * p:KSEC + VW * (p + 1)])

    # ---- attention + interleaved output projection ----
    attn_sb_pool = ctx.enter_context(tc.tile_pool(name="attnsb", bufs=1))
    attn_sb = [attn_sb_pool.tile([P, FREE], BF16, name=f"attnsb{p}")
               for p in range(PAIRS)]
    small_pool = ctx.enter_context(tc.tile_pool(name="small", bufs=4))

    op_tasks = []
    op_stage = []
    op_stage2 = []

    with tc.tile_pool(name="sc_ps", bufs=2, space="PSUM") as sc_ps, \
         tc.tile_pool(name="at_ps", bufs=2, space="PSUM") as at_ps, \
         tc.tile_pool(name="op_ps", bufs=1, space="PSUM") as op_ps, \
         tc.tile_pool(name="e_sb", bufs=4) as e_pool, \
         tc.tile_pool(name="atsb", bufs=4) as atsb_pool:

        def run_op(p_, st):
            op2 = op_ps.tile([P, D], F32, name=f"op{p_}_{st}", tag="op")
            for eb in range(2):
                nc.tensor.matmul(op2[:, FREE * eb:FREE * (eb + 1)],
                                 attn_sb[p_][:, P * st:P * (st + 1)],
                                 wo[p_][:, FREE * eb:FREE * (eb + 1)],
                                 start=True, stop=True)
            nc.vector.tensor_add(osb[st][:], osb[st][:], op2[:])

        def normalize(p, at0, at1):
            # atsb copies (frees the PV banks fast), reciprocal of Z (row
            # 64), GPSIMD broadcast, then the normalizing muls into attn_sb
            atsb0 = atsb_pool.tile([HD + 1, FREE], F32, name=f"as0_{p}",
                                   tag="atsb")
            atsb1 = atsb_pool.tile([HD + 1, FREE], F32, name=f"as1_{p}",
                                   tag="atsb")
            nc.vector.tensor_copy(atsb0[:], at0[0:HD + 1, :])
            nc.vector.tensor_copy(atsb1[:], at1[0:HD + 1, :])
            rz0 = small_pool.tile([1, FREE], F32, name=f"rz0_{p}", tag="rz")
            rz1 = small_pool.tile([1, FREE], F32, name=f"rz1_{p}", tag="rz")
            nc.vector.reciprocal(rz0[:], atsb0[HD:HD + 1, :])
            nc.vector.reciprocal(rz1[:], atsb1[HD:HD + 1, :])
            rzb0 = small_pool.tile([HD, FREE], F32, name=f"rb0_{p}",
                                   tag="rzb")
            rzb1 = small_pool.tile([HD, FREE], F32, name=f"rb1_{p}",
                                   tag="rzb")
            nc.gpsimd.partition_broadcast(rzb0[:], rz0[:])
            nc.gpsimd.partition_broadcast(rzb1[:], rz1[:])
            nc.vector.tensor_mul(attn_sb[p][0:HD, :], atsb0[0:HD, :],
                                 rzb0[:])
            nc.vector.tensor_mul(attn_sb[p][HD:P, :], atsb1[0:HD, :],
                                 rzb1[:])
            # stage outproj tasks one pair behind so attention ACT slack
            # absorbs them
            op_tasks.extend(op_stage)
            op_stage.clear()
            for st in range(SL // P):
                op_stage.append((p, st))

        pv_pending = []

        def attn_v(p, at0, at1, g, e_a, e_b):
            h0, h1 = 2 * p, 2 * p + 1
            for j2 in range(2):
                c = 2 * g + j2
                nc.tensor.matmul(at0[:], vt[c][:, 65 * h0:65 * h0 + P],
                                 e_a[:, FREE * j2:FREE * (j2 + 1)],
                                 start=(c == 0), stop=(c == NKK - 1))
                nc.tensor.matmul(at1[:], vt[c][:, 65 * h1:65 * h1 + P],
                                 e_b[:, FREE * j2:FREE * (j2 + 1)],
                                 start=(c == 0), stop=(c == NKK - 1))

        for p in range(PAIRS):
            at0 = at_ps.tile([P, FREE], F32, name=f"at0_{p}", tag="at")
            at1 = at_ps.tile([P, FREE], F32, name=f"at1_{p}", tag="at")
            for g in range(NKK // 2):
                psA = sc_ps.tile([P, 2 * FREE], F32, name=f"psA{p}_{g}",
                                 tag="sc")
                psB = sc_ps.tile([P, 2 * FREE], F32, name=f"psB{p}_{g}",
                                 tag="sc")
                for j2 in range(2):
                    c = 2 * g + j2
                    nc.tensor.matmul(psA[:, FREE * j2:FREE * (j2 + 1)],
                                     kt_pair[p][0:HD, P * c:P * (c + 1)],
                                     qp[p][0:HD, :], start=True, stop=True)
                    nc.tensor.matmul(psB[:, FREE * j2:FREE * (j2 + 1)],
                                     kt_pair[p][HD:P, P * c:P * (c + 1)],
                                     qp[p][HD:P, :], start=True, stop=True)
                e_a = e_pool.tile([P, 2 * FREE], BF16, name=f"eA{p}_{g}",
                                  tag="e")
                e_b = e_pool.tile([P, 2 * FREE], BF16, name=f"eB{p}_{g}",
                                  tag="e")
                nc.scalar.activation(e_a[:], psA[:], AF.Exp)
                nc.scalar.activation(e_b[:], psB[:], AF.Exp)
                # PV one group behind the exp stream
                if pv_pending:
                    attn_v(*pv_pending.pop())
                pv_pending.append((p, at0, at1, g, e_a, e_b))
                if op_tasks:
                    run_op(*op_tasks.pop(0))
            normalize_args = (p, at0, at1)
            if pv_pending and pv_pending[0][3] == NKK // 2 - 1:
                # flush so the pair's accumulation closes before normalize
                attn_v(*pv_pending.pop())
            normalize(*normalize_args)
        if pv_pending:
            attn_v(*pv_pending.pop())
        op_tasks.extend(op_stage)
        while op_tasks:
            run_op(*op_tasks.pop(0))
        for st in range(SL // P):
            nc.sync.dma_start(out.ap()[P * st:P * (st + 1), :], osb[st][:])

    ctx.close()


_CACHE = {}


def _get_program():
    if "nc" not in _CACHE:
        _CACHE["nc"] = build_program()
    return _CACHE["nc"]


def prep_inputs(input_tensor, qkv_weight, qkv_bias, out_weight, out_bias):
    """Host-side shard + transpose + cast. Returns in_maps for 8 cores."""
    x = np.asarray(input_tensor, np.float32)
    wqkv = np.asarray(qkv_weight, np.float32).copy()
    bq = np.asarray(qkv_bias, np.float32).copy()
    wout = np.asarray(out_weight, np.float32)
    scale = 1.0 / np.sqrt(np.float32(HD))
    wqkv[:D] *= scale
    bq[:D] *= scale
    bf = ml_dtypes.bfloat16
    wqkvT = np.ascontiguousarray(wqkv.T).astype(bf)
    bqk = np.ascontiguousarray(bq[:2 * D].reshape(16, P).T).astype(bf)
    woutT = np.ascontiguousarray(wout.T).astype(bf)
    bout_eff = np.asarray(out_bias, np.float32) + wout @ bq[2 * D:]
    bout_f = np.ascontiguousarray(bout_eff.reshape(1, D)).astype(np.float32)
    xTb = [np.ascontiguousarray(x[b].T).astype(bf) for b in range(B)]
    in_maps = []
    for c in range(N_CORES):
        b, j = c // R, c % R
        xqT = np.ascontiguousarray(xTb[b][:, SL * j:SL * (j + 1)])
        in_maps.append({"xqT": xqT, "wqkvT": wqkvT, "bqk": bqk,
                        "woutT": woutT, "bout": bout_f})
    return in_maps


def kernel(input_tensor, qkv_weight, qkv_bias, out_weight, out_bias,
           **run_kwargs):
    nc = _get_program()
    in_maps = prep_inputs(input_tensor, qkv_weight, qkv_bias, out_weight,
                          out_bias)
    res = run_bass_kernel_spmd(nc, in_maps, core_ids=list(range(N_CORES)),
                               **run_kwargs)
    full = np.empty((B, S, D), np.float32)
    for c in range(N_CORES):
        b, j = c // R, c % R
        full[b, SL * j:SL * (j + 1), :] = res.results[c]["out"]
    if run_kwargs:
        kernel.last_results = res
    return full
